# revision 1
# baseline (speedup 1.0000x reference)
"""NeuralCDE RK4 solver as a Bass/Tile kernel on 8 Trainium2 cores.

Data-parallel over batch: B=1024 -> 128 rows per core (one partition tile).
The 127-step RK4 scan is fully unrolled; per stage:
    mm1 (PE)  : h_psum[128m,128b] = W1z.T @ zT_stage
    relu (ACT): hS = relu(h_psum + bias1(t))     (time channel folded in bias)
    mm2 (PE)  : f_psum[128b,512]  = ones.T@b2 + hS.T @ W2   (accumulated)
    tanh (ACT): fS = tanh(f_psum)
    mul  (DVE): u = fS * g(step,stage)           (g broadcast along h via AP)
    red  (DVE): k_nat[128b,64] = sum_c u
    T    (PE) : k^T accumulated into acc_psum    (RK4 weights pre-folded in g)
    stt  (DVE): z_stage_next = k^T * alpha + zT
State z^T lives in one big SBUF buffer [64, 128*128] (slot per grid point);
slots stream out to DRAM as they finish.
"""

import numpy as np
import ml_dtypes

import concourse.bacc as bacc
import concourse.bass as bass
import concourse.mybir as mybir
from concourse.tile import TileContext
from concourse.bass_utils import run_bass_kernel_spmd

F32 = mybir.dt.float32
F32R = mybir.dt.float32r
BF16 = mybir.dt.bfloat16
FP16 = mybir.dt.float16
B = 1024
L = 128
C_IN = 8
HID = 64
MLP_H = 128
INIT_H = 20
NSTEP = L - 1  # 127
NCORES = 8
BL = B // NCORES  # 128 batch rows per core

_CACHE: dict = {}


def _flags():
    import os
    return (
        os.environ.get("K_T_F32R", "0") == "1",
        os.environ.get("K_MM2_F32R", "1") == "1",
        os.environ.get("K_MUL_BF16", "1") == "1",
        os.environ.get("K_MM1_F32R", "1") == "1",
        os.environ.get("K_WARM", "0") == "1",
        os.environ.get("K_FP16_PATH", "1") == "1",
        os.environ.get("K_T_FP16", "0") == "1",
        os.environ.get("K_MM1_SPLIT", "1") == "1",
    )


def _build(nstep: int, with_b2: bool):
    import time as _time

    t_f32r, mm2_f32r, mul_bf16, mm1_f32r, warm, fp16_path, t_fp16, mm1_split = _flags()
    TD = F32R if t_f32r else F32
    if t_fp16:
        TD = FP16
    SD = F32R if mm1_f32r else F32
    MD = F32R if mm2_f32r else F32
    UD = BF16 if mul_bf16 else F32
    if fp16_path:
        MD = FP16
        UD = FP16
    t0 = _time.time()
    nc = bacc.Bacc()
    g_in = nc.dram_tensor("g", [BL, nstep * 3 * C_IN], UD, kind="ExternalInput")
    b1_in = nc.dram_tensor("bias1", [MLP_H, nstep * 3], F32, kind="ExternalInput")
    w1z_in = nc.dram_tensor("w1z", [HID, MLP_H], SD, kind="ExternalInput")
    w2_in = nc.dram_tensor("w2", [MLP_H, HID * C_IN], MD, kind="ExternalInput")
    b2_in = nc.dram_tensor("b2r", [1, HID * C_IN], MD, kind="ExternalInput")
    ones_in = nc.dram_tensor("onesr", [1, BL], MD, kind="ExternalInput")
    id_in = nc.dram_tensor("ident", [BL, BL], TD, kind="ExternalInput")
    z0t_in = nc.dram_tensor("z0t", [HID, BL], SD, kind="ExternalInput")
    w1zh_in = nc.dram_tensor("w1zh", [HID, MLP_H], FP16, kind="ExternalInput")
    zs_out = nc.dram_tensor(
        "zs", [HID, (nstep + 1) * BL], F32, kind="ExternalOutput"
    )

    NF = HID * C_IN  # 512
    with TileContext(nc) as tc:
        with (
            tc.tile_pool(name="const", bufs=1) as cp,
            tc.tile_pool(name="zst", bufs=1) as zp,
            tc.tile_pool(name="hs", bufs=3) as hp,
            tc.tile_pool(name="fs", bufs=2) as fp,
            tc.tile_pool(name="us", bufs=2) as up,
            tc.tile_pool(name="ks", bufs=3) as kp,
            tc.tile_pool(name="zc", bufs=3) as zcp,
            tc.tile_pool(name="kh", bufs=2) as khp,
            tc.tile_pool(name="ph", bufs=(4 if mm1_split else 2), space="PSUM") as ph,
            tc.tile_pool(name="pf", bufs=2, space="PSUM") as pf,
            tc.tile_pool(name="pacc", bufs=(1 if mm1_split else 2), space="PSUM") as pacc,
            tc.tile_pool(name="pks", bufs=1, space="PSUM") as pks,
            tc.tile_pool(name="pfill", bufs=1, space="PSUM") as pfill,
        ):
            gS = cp.tile([BL, nstep * 3 * C_IN], UD)
            b1S = cp.tile([MLP_H, nstep * 3], F32)
            w1zS = cp.tile([HID, MLP_H], SD)
            w1zH = cp.tile([HID, MLP_H], FP16)
            w2S = cp.tile([MLP_H, NF], MD)
            b2S = cp.tile([1, NF], MD)
            onesS = cp.tile([1, BL], MD)
            idS = cp.tile([BL, BL], TD)
            zall = zp.tile([HID, (nstep + 1) * BL], SD)
            if warm:
                wt = cp.tile([BL, BL], BF16, name="wt")
                nc.vector.memset(wt[:], 0.0)

            nc.sync.dma_start(out=gS[:], in_=g_in[:])
            nc.sync.dma_start(out=b1S[:], in_=b1_in[:])
            nc.sync.dma_start(out=w1zS[:], in_=w1z_in[:])
            nc.sync.dma_start(out=w1zH[:], in_=w1zh_in[:])
            nc.sync.dma_start(out=w2S[:], in_=w2_in[:])
            nc.sync.dma_start(out=b2S[:], in_=b2_in[:])
            nc.sync.dma_start(out=onesS[:], in_=ones_in[:])
            nc.sync.dma_start(out=idS[:], in_=id_in[:])
            nc.sync.dma_start(out=zall[:, 0:BL], in_=z0t_in[:])
            nc.sync.dma_start(out=zs_out[:, 0:BL], in_=z0t_in[:].bitcast(F32))

            if warm:
                wp = pfill.tile([BL, BL], F32, tag="fl", name="wp")
                for _ in range(48):
                    nc.tensor.matmul(
                        wp[:], lhsT=wt[:], rhs=wt[:], start=True, stop=True
                    )
            CLS = (0, 1, 1, 2)
            ALPHA = (0.5, 0.25, 0.5, 1.0 / 6.0)
            prev_accP = None
            for step in range(nstep):
                zT = zall[:, step * BL : (step + 1) * BL]
                cur = zT
                accP = None
                h_tiles = []
                if mm1_split:
                    zT_prev = zall[:, (step - 1) * BL : step * BL]
                    for s in range(4):
                        h_ps_s = ph.tile([MLP_H, BL], F32, tag="hps", name="hps")
                        has_b = not (step == 0 and s == 0)
                        nc.tensor.matmul(
                            h_ps_s[:],
                            lhsT=w1zS[:],
                            rhs=(zT_prev if (s == 0 and step > 0) else zT),
                            start=True,
                            stop=not has_b,
                        )
                        h_tiles.append(h_ps_s)
                for s in range(4):
                    col = step * 3 + CLS[s]
                    if mm1_split:
                        h_ps = h_tiles[s]
                        has_b = not (step == 0 and s == 0)
                        if has_b:
                            if s == 0:
                                ksrc, alpha_b = prev_accP, 1.0 / 6.0
                            elif s == 1:
                                ksrc, alpha_b = accP, 0.5
                            else:
                                ksrc, alpha_b = prev_ksP, 0.25 if s == 2 else 0.5
                            kh = khp.tile([HID, BL], FP16, tag="kh", name="kh")
                            nc.vector.tensor_scalar_mul(kh[:], ksrc[:], alpha_b)
                            nc.tensor.matmul(
                                h_ps[:],
                                lhsT=w1zH[:],
                                rhs=kh[:],
                                start=False,
                                stop=True,
                            )
                    else:
                        h_ps = ph.tile([MLP_H, BL], F32, tag="hps")
                        nc.tensor.matmul(
                            h_ps[:],
                            lhsT=w1zS[:],
                            rhs=cur,
                            start=True,
                            stop=True,
                        )
                    hS = hp.tile([MLP_H, BL], MD, tag="hs")
                    nc.vector.tensor_scalar(
                        hS[:],
                        h_ps[:],
                        b1S[:, col : col + 1],
                        0.0,
                        op0=mybir.AluOpType.add,
                        op1=mybir.AluOpType.max,
                    )
                    f_ps = pf.tile([BL, NF], F32, tag="fps")
                    if with_b2:
                        nc.tensor.matmul(
                            f_ps[:],
                            lhsT=onesS[:],
                            rhs=b2S[:],
                            start=True,
                            stop=False,
                        )
                    nc.tensor.matmul(
                        f_ps[:],
                        lhsT=hS[:],
                        rhs=w2S[:],
                        start=not with_b2,
                        stop=True,
                    )
                    fS = fp.tile([BL, NF], UD, tag="fs")
                    nc.scalar.activation(
                        fS[:], f_ps[:], mybir.ActivationFunctionType.Tanh
                    )
                    if warm:
                        fl1 = pfill.tile([BL, BL], F32, tag="fl", name="fl1")
                        nc.tensor.matmul(
                            fl1[:],
                            lhsT=fS[:, 0:BL],
                            rhs=fS[:, 0:BL],
                            start=True,
                            stop=True,
                        )
                    u = up.tile([BL, NF], UD, tag="u")
                    f3 = fS[:].rearrange("p (h c) -> p h c", c=C_IN)
                    u3 = u[:].rearrange("p (h c) -> p h c", c=C_IN)
                    gv = (
                        gS[:, col * C_IN : (col + 1) * C_IN]
                        .unsqueeze(1)
                        .broadcast_to((BL, HID, C_IN))
                    )
                    nc.vector.tensor_tensor(
                        out=u3, in0=f3, in1=gv, op=mybir.AluOpType.mult
                    )
                    if warm:
                        fl2 = pfill.tile([BL, BL], F32, tag="fl", name="fl2")
                        nc.tensor.matmul(
                            fl2[:],
                            lhsT=u[:, 0:BL],
                            rhs=u[:, 0:BL],
                            start=True,
                            stop=True,
                        )
                    kn = kp.tile([BL, HID], TD, tag="kn")
                    with nc.allow_low_precision("k reduce output precision"):
                        nc.vector.tensor_reduce(
                            kn[:], u3, axis=mybir.AxisListType.X, op=mybir.AluOpType.add
                        )
                    if s == 0:
                        accP = pacc.tile([HID, BL], TD, tag="acc")
                        nc.tensor.matmul(
                            accP[:],
                            lhsT=kn[:],
                            rhs=idS[:],
                            is_transpose=True,
                            start=True,
                            stop=True,
                        )
                        src = accP
                    elif s in (1, 2):
                        ksP = pks.tile([HID, BL], TD, tag="ks")
                        nc.tensor.matmul(
                            ksP[:],
                            lhsT=kn[:],
                            rhs=idS[:],
                            is_transpose=True,
                            start=True,
                            stop=True,
                        )
                        nc.tensor.matmul(
                            accP[:],
                            lhsT=kn[:],
                            rhs=idS[:],
                            is_transpose=True,
                            start=False,
                            stop=True,
                            skip_group_check=True,
                        )
                        src = ksP
                    else:
                        nc.tensor.matmul(
                            accP[:],
                            lhsT=kn[:],
                            rhs=idS[:],
                            is_transpose=True,
                            start=False,
                            stop=True,
                            skip_group_check=True,
                        )
                        src = accP
                    if s in (1, 2):
                        prev_ksP = ksP
                    if (not mm1_split) or s == 3:
                        if s < 3:
                            out_ap = zcp.tile([HID, BL], SD, tag="zc", name="zc")[:]
                        else:
                            out_ap = zall[:, (step + 1) * BL : (step + 2) * BL]
                        nc.vector.scalar_tensor_tensor(
                            out=out_ap,
                            in0=src[:],
                            scalar=ALPHA[s],
                            in1=zT,
                            op0=mybir.AluOpType.mult,
                            op1=mybir.AluOpType.add,
                        )
                        if s < 3:
                            cur = out_ap
                prev_accP = accP
                nc.sync.dma_start(
                    out=zs_out[:, (step + 1) * BL : (step + 2) * BL],
                    in_=zall[:, (step + 1) * BL : (step + 2) * BL].bitcast(F32),
                )
    import sys

    print(f"[kernel] tile trace+schedule: {_time.time()-t0:.1f}s", file=sys.stderr)
    t1 = _time.time()
    nc.finalize()
    print(f"[kernel] finalize: {_time.time()-t1:.1f}s", file=sys.stderr)
    return nc


def _get_nc(nstep: int, with_b2: bool):
    key = (nstep, with_b2) + _flags()
    if key not in _CACHE:
        _CACHE[key] = _build(nstep, with_b2)
    return _CACHE[key]


def _host_prep(coeffs, Wi1, bi1, Wi2, bi2, W1, b1, W2, b2, nstep: int):
    coeffs = np.asarray(coeffs, dtype=np.float32)
    a = coeffs[:, :, 0:8]
    b = coeffs[:, :, 8:16]
    c = coeffs[:, :, 16:24]
    d = coeffs[:, :, 24:32]

    X0 = a[:, 0]
    z0 = np.tanh(
        np.maximum(X0 @ Wi1 + bi1, 0.0).astype(np.float32) @ Wi2 + bi2
    ).astype(np.float32)

    g = np.empty((B, nstep, 3, C_IN), dtype=np.float32)
    g[:, :, 0] = b[:, :nstep]
    g[:, :, 1] = 2.0 * b[:, :nstep] + 2.0 * c[:, :nstep] + 1.5 * d[:, :nstep]
    # stage-4 derivative: dXdt at t=i+1
    last = NSTEP - 1  # 126 in full problem
    for i in range(nstep):
        if i < last:
            g[:, i, 2] = b[:, i + 1]
        else:
            g[:, i, 2] = b[:, i] + 2.0 * c[:, i] + 3.0 * d[:, i]

    tcols = np.empty((nstep, 3), dtype=np.float32)
    tcols[:, 0] = np.arange(nstep, dtype=np.float32)
    tcols[:, 1] = tcols[:, 0] + 0.5
    tcols[:, 2] = tcols[:, 0] + 1.0
    # bias1[m, step*3+cls] = b1[m] + t * W1[0, m]
    bias1 = (
        b1[None, None, :] + tcols[:, :, None] * W1[0][None, None, :]
    ).astype(np.float32)
    bias1 = bias1.reshape(nstep * 3, MLP_H).T.copy()  # [128, nstep*3]

    wdt = np.float16 if _flags()[5] else np.float32
    shared = {
        "bias1": bias1,
        "w1z": np.ascontiguousarray(W1[1:], dtype=np.float32),
        "w1zh": np.ascontiguousarray(W1[1:], dtype=np.float16),
        "w2": np.ascontiguousarray(W2, dtype=wdt),
        "b2r": np.ascontiguousarray(b2[None, :], dtype=wdt),
        "onesr": np.ones((1, BL), dtype=wdt),
        "ident": np.eye(
            BL, dtype=np.float16 if _flags()[6] else np.float32
        ),
    }
    in_maps = []
    for core in range(NCORES):
        sl = slice(core * BL, (core + 1) * BL)
        m = dict(shared)
        f = _flags()
        gdt = np.float16 if f[5] else (ml_dtypes.bfloat16 if f[2] else np.float32)
        m["g"] = np.ascontiguousarray(
            g[sl].reshape(BL, nstep * 3 * C_IN).astype(gdt)
        )
        m["z0t"] = np.ascontiguousarray(z0[sl].T)
        in_maps.append(m)
    return in_maps, z0


def kernel(coeffs, Wi1, bi1, Wi2, bi2, W1, b1, W2, b2, _nstep: int = NSTEP,
           _trace: bool = False):
    import time as _time
    import sys

    nstep = _nstep
    with_b2 = bool(np.any(np.asarray(b2)))
    nc = _get_nc(nstep, with_b2)
    in_maps, _ = _host_prep(
        coeffs, Wi1, bi1, Wi2, bi2, W1, b1, W2, b2, nstep
    )
    t0 = _time.time()
    res = run_bass_kernel_spmd(nc, in_maps, list(range(NCORES)), trace=_trace)
    print(f"[kernel] spmd run (compile+exec): {_time.time()-t0:.1f}s", file=sys.stderr)
    out = np.empty((B, nstep + 1, HID), dtype=np.float32)
    for core in range(NCORES):
        zs = res.results[core]["zs"].reshape(HID, nstep + 1, BL)
        out[core * BL : (core + 1) * BL] = zs.transpose(2, 1, 0)
    if _trace:
        kernel.last_results = res
    return out



# revision 2
# speedup vs baseline: 1.6220x; 1.6220x over previous
"""NeuralCDE RK4 solver as a Bass/Tile kernel on 8 Trainium2 cores.

Data-parallel over batch: B=1024 -> 128 rows per core. The 127-step RK4
scan is fully unrolled. The key restructuring vs a naive lowering: the
MLP output f is produced in a TRANSPOSED layout fT[(h',c), (j,b)] via 4
column-chunked mm2 matmuls, so the einsum k[b,h] = sum_c f[b,h,c]*g[b,c]
fuses into the NEXT stage's mm1 using replicated weights
W1Rep[(h',c),m] = alpha*W1[16j+h',m]. This removes the tensor_reduce,
the PE transpose, and the alpha-scale from the per-stage critical chain:

    per stage:  mul (DVE)  : u = tanh(fT) * gRep   [128, 512] fp16
                mm1  (PE)  : h_ps = W1^T z  (base, early)
                             + sum_j W1RepA_j^T u_j   (4 accum matmuls)
                relu (DVE) : hS = relu(h_ps + bias1(t))  (t folded in bias)
                mm2  (PE)  : fT_ps[:, j*128:...] = W2_j^T @ hS  (4 matmuls)
                tanh (ACT) : fS = tanh(fT_ps)

The z-update k-sums come from small side matmuls (S selection matrices,
RK4 weights folded in) accumulating into accP[64,128] PSUM off the
critical path; z' = z + accP via one DVE op, fp16 copy via ACT.
g (the dX/dt factors, partition-replicated) streams from DRAM per step.
"""

import numpy as np
import ml_dtypes

import concourse.bacc as bacc
import concourse.bass as bass
import concourse.mybir as mybir
from concourse.tile import TileContext
from concourse.bass_utils import run_bass_kernel_spmd

F32 = mybir.dt.float32
FP16 = mybir.dt.float16
B = 1024
L = 128
C_IN = 8
HID = 64
MLP_H = 128
NSTEP = L - 1  # 127
NCORES = 8
BL = B // NCORES  # 128 batch rows per core
NF = HID * C_IN  # 512

_CACHE: dict = {}


def _build(nstep: int, with_b2: bool):
    import time as _time
    import sys

    t0 = _time.time()
    nc = bacc.Bacc()
    grep_in = nc.dram_tensor("grep", [BL, nstep * 3 * BL], FP16, kind="ExternalInput")
    b1_in = nc.dram_tensor("bias1", [MLP_H, nstep * 3], F32, kind="ExternalInput")
    w1z_in = nc.dram_tensor("w1z", [HID, MLP_H], FP16, kind="ExternalInput")
    w1a_in = nc.dram_tensor("w1a", [MLP_H, 4 * MLP_H], FP16, kind="ExternalInput")
    w1f_in = nc.dram_tensor("w1f", [MLP_H, 4 * MLP_H], FP16, kind="ExternalInput")
    s6_in = nc.dram_tensor("s6", [MLP_H, 4 * HID], FP16, kind="ExternalInput")
    s3_in = nc.dram_tensor("s3", [MLP_H, 4 * HID], FP16, kind="ExternalInput")
    w2_in = nc.dram_tensor("w2", [MLP_H, NF], FP16, kind="ExternalInput")
    z0t_in = nc.dram_tensor("z0t", [HID, BL], F32, kind="ExternalInput")
    z0h_in = nc.dram_tensor("z0h", [HID, BL], FP16, kind="ExternalInput")
    if with_b2:
        b2c_in = nc.dram_tensor("b2c", [4, MLP_H], FP16, kind="ExternalInput")
        jsel_in = nc.dram_tensor("jsel", [4, NF], FP16, kind="ExternalInput")
    zs_out = nc.dram_tensor("zs", [HID, (nstep + 1) * BL], F32, kind="ExternalOutput")

    CLS = (0, 1, 1, 2)
    Tanh = mybir.ActivationFunctionType.Tanh
    Copy = mybir.ActivationFunctionType.Copy

    with TileContext(nc) as tc:
        with (
            tc.tile_pool(name="const", bufs=1) as cp,
            tc.tile_pool(name="zst", bufs=1) as zp,
            tc.tile_pool(name="g", bufs=6) as gp,
            tc.tile_pool(name="hs", bufs=3) as hp,
            tc.tile_pool(name="fs", bufs=2) as fp,
            tc.tile_pool(name="us", bufs=3) as up,
            tc.tile_pool(name="zh", bufs=3) as zhp,
            tc.tile_pool(name="kh", bufs=2) as khp,
            tc.tile_pool(name="ph", bufs=3, space="PSUM") as ph,
            tc.tile_pool(name="pf", bufs=2, space="PSUM") as pf,
            tc.tile_pool(name="pa", bufs=2, space="PSUM") as pa,
        ):
            b1S = cp.tile([MLP_H, nstep * 3], F32)
            w1zS = cp.tile([HID, MLP_H], FP16)
            w1aS = cp.tile([MLP_H, 4 * MLP_H], FP16)
            w1fS = cp.tile([MLP_H, 4 * MLP_H], FP16)
            s6S = cp.tile([MLP_H, 4 * HID], FP16)
            s3S = cp.tile([MLP_H, 4 * HID], FP16)
            w2S = cp.tile([MLP_H, NF], FP16)
            zall = zp.tile([HID, (nstep + 1) * BL], F32)

            nc.sync.dma_start(out=b1S[:], in_=b1_in[:])
            nc.sync.dma_start(out=w1zS[:], in_=w1z_in[:])
            nc.sync.dma_start(out=w1aS[:], in_=w1a_in[:])
            nc.sync.dma_start(out=w1fS[:], in_=w1f_in[:])
            nc.sync.dma_start(out=s6S[:], in_=s6_in[:])
            nc.sync.dma_start(out=s3S[:], in_=s3_in[:])
            nc.sync.dma_start(out=w2S[:], in_=w2_in[:])
            if with_b2:
                b2cS = cp.tile([4, MLP_H], FP16)
                jselS = cp.tile([4, NF], FP16)
                nc.sync.dma_start(out=b2cS[:], in_=b2c_in[:])
                nc.sync.dma_start(out=jselS[:], in_=jsel_in[:])
            nc.sync.dma_start(out=zall[:, 0:BL], in_=z0t_in[:])
            nc.sync.dma_start(out=zs_out[:, 0:BL], in_=z0t_in[:])
            zh_prev = zhp.tile([HID, BL], FP16, name="zh0")
            nc.sync.dma_start(out=zh_prev[:], in_=z0h_in[:])
            zh_cur = zh_prev  # z_n fp16 for bases of stages 2..4

            kh_prev = None
            accP = None
            u_prev = None
            sred_pending = None
            for step in range(nstep):
                gslot = gp.tile([BL, 3 * BL], FP16, tag="g")
                nc.sync.dma_start(
                    out=gslot[:], in_=grep_in[:, step * 3 * BL : (step + 1) * 3 * BL]
                )
                accP_prev = accP
                accP = pa.tile([HID, BL], F32, tag="acc")
                for s in range(4):
                    col = step * 3 + CLS[s]
                    h_ps = ph.tile([MLP_H, BL], F32, tag="hps")
                    if s == 0:
                        has_acc = kh_prev is not None
                        nc.tensor.matmul(
                            h_ps[:],
                            lhsT=w1zS[:],
                            rhs=zh_prev[:],
                            start=True,
                            stop=not has_acc,
                        )
                        if has_acc:
                            nc.tensor.matmul(
                                h_ps[:],
                                lhsT=w1zS[:],
                                rhs=kh_prev[:],
                                start=False,
                                stop=True,
                            )
                    else:
                        wrep = w1fS if s == 3 else w1aS
                        nc.tensor.matmul(
                            h_ps[:],
                            lhsT=w1zS[:],
                            rhs=zh_cur[:],
                            start=True,
                            stop=False,
                        )
                        for j in range(4):
                            nc.tensor.matmul(
                                h_ps[:],
                                lhsT=wrep[:, j * MLP_H : (j + 1) * MLP_H],
                                rhs=u_prev[:, j * BL : (j + 1) * BL],
                                start=False,
                                stop=(j == 3),
                            )
                    # side matmuls for the z-update of the previous stage
                    if sred_pending is not None:
                        sW, sU, sAcc, s_start, s_stop = sred_pending
                        for j in range(4):
                            nc.tensor.matmul(
                                sAcc[:],
                                lhsT=sW[:, j * HID : (j + 1) * HID],
                                rhs=sU[:, j * BL : (j + 1) * BL],
                                start=(s_start and j == 0),
                                stop=(s_stop and j == 3),
                                skip_group_check=not (s_start and j == 0),
                            )
                        sred_pending = None

                    hS = hp.tile([MLP_H, BL], FP16, tag="hs")
                    nc.vector.tensor_scalar(
                        hS[:],
                        h_ps[:],
                        b1S[:, col : col + 1],
                        0.0,
                        op0=mybir.AluOpType.add,
                        op1=mybir.AluOpType.max,
                    )
                    f_ps = pf.tile([MLP_H, NF], F32, tag="fps")
                    if with_b2:
                        nc.tensor.matmul(
                            f_ps[:], lhsT=b2cS[:], rhs=jselS[:], start=True, stop=False
                        )
                    for j in range(4):
                        nc.tensor.matmul(
                            f_ps[:, j * BL : (j + 1) * BL],
                            lhsT=w2S[:, j * BL : (j + 1) * BL],
                            rhs=hS[:],
                            start=not with_b2,
                            stop=True,
                            skip_group_check=with_b2,
                        )
                    fS = fp.tile([MLP_H, NF], FP16, tag="fs")
                    nc.scalar.activation(fS[:], f_ps[:], Tanh)
                    u = up.tile([MLP_H, NF], FP16, tag="u")
                    u3 = u[:].rearrange("p (j b) -> p j b", j=4)
                    f3 = fS[:].rearrange("p (j b) -> p j b", j=4)
                    gv = (
                        gslot[:, CLS[s] * BL : (CLS[s] + 1) * BL]
                        .unsqueeze(1)
                        .broadcast_to((BL, 4, BL))
                    )
                    nc.vector.tensor_tensor(
                        out=u3, in0=f3, in1=gv, op=mybir.AluOpType.mult
                    )
                    sred_pending = (
                        s6S if s in (0, 3) else s3S,
                        u,
                        accP,
                        s == 0,
                        s == 3,
                    )
                    u_prev = u
                # flush stage-3 side matmuls now (kh depends on them)
                sW, sU, sAcc, s_start, s_stop = sred_pending
                for j in range(4):
                    nc.tensor.matmul(
                        sAcc[:],
                        lhsT=sW[:, j * HID : (j + 1) * HID],
                        rhs=sU[:, j * BL : (j + 1) * BL],
                        start=False,
                        stop=(j == 3),
                        skip_group_check=True,
                    )
                sred_pending = None
                kh_prev = khp.tile([HID, BL], FP16, tag="kh")
                nc.vector.tensor_scalar_mul(kh_prev[:], accP[:], 1.0)
                zh_prev = zh_cur
                cur_sl = zall[:, step * BL : (step + 1) * BL]
                nxt_sl = zall[:, (step + 1) * BL : (step + 2) * BL]
                nc.vector.scalar_tensor_tensor(
                    out=nxt_sl,
                    in0=accP[:],
                    scalar=1.0,
                    in1=cur_sl,
                    op0=mybir.AluOpType.mult,
                    op1=mybir.AluOpType.add,
                )
                zh_cur = zhp.tile([HID, BL], FP16, tag="zh")
                nc.scalar.activation(zh_cur[:], nxt_sl, Copy)
                nc.gpsimd.dma_start(
                    out=zs_out[:, (step + 1) * BL : (step + 2) * BL], in_=nxt_sl
                )

    print(f"[kernel] tile trace+schedule: {_time.time()-t0:.1f}s", file=sys.stderr)
    t1 = _time.time()
    nc.finalize()
    print(f"[kernel] finalize: {_time.time()-t1:.1f}s", file=sys.stderr)
    return nc


def _get_nc(nstep: int, with_b2: bool):
    key = (nstep, with_b2)
    if key not in _CACHE:
        _CACHE[key] = _build(nstep, with_b2)
    return _CACHE[key]


def _host_prep(coeffs, Wi1, bi1, Wi2, bi2, W1, b1, W2, b2, nstep: int):
    coeffs = np.asarray(coeffs, dtype=np.float32)
    a = coeffs[:, :, 0:8]
    b = coeffs[:, :, 8:16]
    c = coeffs[:, :, 16:24]
    d = coeffs[:, :, 24:32]

    X0 = a[:, 0]
    z0 = np.tanh(
        np.maximum(X0 @ Wi1 + bi1, 0.0).astype(np.float32) @ Wi2 + bi2
    ).astype(np.float32)

    # g[b, i, cls, c] = dX/dt at stage times (cls 0: t=i, 1: t=i+.5, 2: t=i+1)
    g = np.empty((B, nstep, 3, C_IN), dtype=np.float32)
    g[:, :, 0] = b[:, :nstep]
    g[:, :, 1] = b[:, :nstep] + c[:, :nstep] + 0.75 * d[:, :nstep]
    for i in range(nstep):
        if i + 1 < L - 1:
            g[:, i, 2] = b[:, i + 1]
        else:
            g[:, i, 2] = b[:, i] + 2.0 * c[:, i] + 3.0 * d[:, i]

    tcols = np.empty((nstep, 3), dtype=np.float32)
    tcols[:, 0] = np.arange(nstep, dtype=np.float32)
    tcols[:, 1] = tcols[:, 0] + 0.5
    tcols[:, 2] = tcols[:, 0] + 1.0
    bias1 = (
        b1[None, None, :] + tcols[:, :, None] * W1[0][None, None, :]
    ).astype(np.float32)
    bias1 = bias1.reshape(nstep * 3, MLP_H).T.copy()

    w1rep = np.repeat(np.asarray(W1[1:], np.float32), C_IN, axis=0)  # [512, 128]
    w1a = np.concatenate(
        [0.5 * w1rep[j * MLP_H : (j + 1) * MLP_H] for j in range(4)], axis=1
    )
    w1f = np.concatenate(
        [w1rep[j * MLP_H : (j + 1) * MLP_H] for j in range(4)], axis=1
    )
    sfull = np.repeat(np.eye(HID, dtype=np.float32), C_IN, axis=0)  # [512, 64]
    s6 = np.concatenate(
        [(1.0 / 6.0) * sfull[j * MLP_H : (j + 1) * MLP_H] for j in range(4)], axis=1
    )
    s3 = np.concatenate(
        [(1.0 / 3.0) * sfull[j * MLP_H : (j + 1) * MLP_H] for j in range(4)], axis=1
    )

    with_b2 = bool(np.any(np.asarray(b2)))
    shared = {
        "bias1": bias1,
        "w1z": np.ascontiguousarray(W1[1:], dtype=np.float16),
        "w1a": np.ascontiguousarray(w1a, dtype=np.float16),
        "w1f": np.ascontiguousarray(w1f, dtype=np.float16),
        "s6": np.ascontiguousarray(s6, dtype=np.float16),
        "s3": np.ascontiguousarray(s3, dtype=np.float16),
        "w2": np.ascontiguousarray(W2, dtype=np.float16),
    }
    if with_b2:
        shared["b2c"] = np.ascontiguousarray(
            np.asarray(b2, np.float32).reshape(4, MLP_H), dtype=np.float16
        )
        shared["jsel"] = np.ascontiguousarray(
            np.kron(np.eye(4, dtype=np.float32), np.ones((1, BL), np.float32)),
            dtype=np.float16,
        )

    in_maps = []
    for core in range(NCORES):
        sl = slice(core * BL, (core + 1) * BL)
        m = dict(shared)
        gc = g[sl]  # [BL, nstep, 3, 8]
        arr = gc.transpose(3, 1, 2, 0)  # [8, nstep, 3, BL]
        rep = np.tile(arr, (MLP_H // C_IN, 1, 1, 1))  # [128, nstep, 3, BL]
        m["grep"] = np.ascontiguousarray(
            rep.reshape(MLP_H, nstep * 3 * BL), dtype=np.float16
        )
        z0t = np.ascontiguousarray(z0[sl].T)
        m["z0t"] = z0t
        m["z0h"] = np.ascontiguousarray(z0t, dtype=np.float16)
        in_maps.append(m)
    return in_maps, with_b2


def kernel(coeffs, Wi1, bi1, Wi2, bi2, W1, b1, W2, b2, _nstep: int = NSTEP,
           _trace: bool = False):
    import time as _time
    import sys

    nstep = _nstep
    in_maps, with_b2 = _host_prep(
        coeffs, Wi1, bi1, Wi2, bi2, W1, b1, W2, b2, nstep
    )
    nc = _get_nc(nstep, with_b2)
    t0 = _time.time()
    res = run_bass_kernel_spmd(nc, in_maps, list(range(NCORES)), trace=_trace)
    print(f"[kernel] spmd run (compile+exec): {_time.time()-t0:.1f}s", file=sys.stderr)
    out = np.empty((B, nstep + 1, HID), dtype=np.float32)
    for core in range(NCORES):
        zs = res.results[core]["zs"].reshape(HID, nstep + 1, BL)
        out[core * BL : (core + 1) * BL] = zs.transpose(2, 1, 0)
    if _trace:
        kernel.last_results = res
    return out


# revision 15
# speedup vs baseline: 1.8318x; 1.1293x over previous
"""NeuralCDE RK4 solver as a Bass/Tile kernel on 8 Trainium2 cores.

Data-parallel over batch: B=1024 -> 128 rows per core. The 127-step RK4
scan is fully unrolled. Two key restructurings vs a naive lowering:

1. The MLP output f is produced in a TRANSPOSED layout fT[(h',c), (j,b)]
   via 4 column-chunked mm2 matmuls, so the einsum
   k[b,h] = sum_c f[b,h,c]*g[b,c] fuses into the NEXT stage's mm1 using
   replicated weights W1Rep[(h',c),m] = alpha*W1[16j+h',m]. This removes
   the tensor_reduce, the PE transpose, and the alpha-scale from the
   per-stage critical chain:
       mul (DVE) -> 4x mm1acc (PE) -> relu (DVE) -> 4x mm2 (PE) -> tanh (ACT)
2. The 128 batch rows per core are split into two 64-row halves whose
   serial chains interleave on the engines (software pipelining), roughly
   halving the per-stage latency.

The z-update k-sums come from small side matmuls (S selection matrices,
RK4 weights folded in) accumulating into accP[64,128] PSUM off the
critical path; z' = z + accP via one DVE op, fp16 copy via ACT.
g (the dX/dt factors, partition-replicated) streams from DRAM per step.
"""

import numpy as np
import ml_dtypes

import concourse.bacc as bacc
import concourse.bass as bass
import concourse.mybir as mybir
from concourse.tile import TileContext
from concourse.bass_utils import run_bass_kernel_spmd

F32 = mybir.dt.float32
FP16 = mybir.dt.float16
B = 1024
L = 128
C_IN = 8
HID = 64
MLP_H = 128
NSTEP = L - 1  # 127
NCORES = 8
BL = B // NCORES  # 128 batch rows per core
NF = HID * C_IN  # 512
NH = 2  # batch halves per core (software pipeline)
HB = BL // NH  # 64

_CACHE: dict = {}


def _build(nstep: int, with_b2: bool):
    import time as _time
    import sys

    t0 = _time.time()
    nc = bacc.Bacc()
    grep_in = nc.dram_tensor("grep", [BL, nstep * 3 * BL], FP16, kind="ExternalInput")
    b1_in = nc.dram_tensor("bias1", [MLP_H, nstep * 3], F32, kind="ExternalInput")
    w1z_in = nc.dram_tensor("w1z", [HID, MLP_H], FP16, kind="ExternalInput")
    w1a_in = nc.dram_tensor("w1a", [MLP_H, 4 * MLP_H], FP16, kind="ExternalInput")
    w1f_in = nc.dram_tensor("w1f", [MLP_H, 4 * MLP_H], FP16, kind="ExternalInput")
    s6_in = nc.dram_tensor("s6", [MLP_H, 4 * HID], FP16, kind="ExternalInput")
    s3_in = nc.dram_tensor("s3", [MLP_H, 4 * HID], FP16, kind="ExternalInput")
    w2_in = nc.dram_tensor("w2", [MLP_H, NF], FP16, kind="ExternalInput")
    z0t_in = nc.dram_tensor("z0t", [HID, BL], F32, kind="ExternalInput")
    z0h_in = nc.dram_tensor("z0h", [HID, BL], FP16, kind="ExternalInput")
    if with_b2:
        b2c_in = nc.dram_tensor("b2c", [4, MLP_H], FP16, kind="ExternalInput")
        jsel_in = nc.dram_tensor("jsel", [4, 4 * HB], FP16, kind="ExternalInput")
    zs_out = nc.dram_tensor("zs", [HID, (nstep + 1) * BL], F32, kind="ExternalOutput")

    CLS = (0, 1, 1, 2)
    Tanh = mybir.ActivationFunctionType.Tanh
    Copy = mybir.ActivationFunctionType.Copy
    HNF = 4 * HB  # 256, per-half f width

    with TileContext(nc) as tc:
        with (
            tc.tile_pool(name="const", bufs=1) as cp,
            tc.tile_pool(name="zst", bufs=1) as zp,
            tc.tile_pool(name="g", bufs=6) as gp,
            tc.tile_pool(name="hs", bufs=3 * NH) as hp,
            tc.tile_pool(name="fs", bufs=2 * NH) as fp,
            tc.tile_pool(name="us", bufs=3 * NH) as up,
            tc.tile_pool(name="zh", bufs=3) as zhp,
            tc.tile_pool(name="kh", bufs=2) as khp,
            tc.tile_pool(name="ph", bufs=2, space="PSUM") as ph,
            tc.tile_pool(name="pf", bufs=1, space="PSUM") as pf,
            tc.tile_pool(name="pa", bufs=2, space="PSUM") as pa,
        ):
            b1S = cp.tile([MLP_H, nstep * 3], F32)
            w1zS = cp.tile([HID, MLP_H], FP16)
            w1aS = cp.tile([MLP_H, 4 * MLP_H], FP16)
            w1fS = cp.tile([MLP_H, 4 * MLP_H], FP16)
            s6S = cp.tile([MLP_H, 4 * HID], FP16)
            s3S = cp.tile([MLP_H, 4 * HID], FP16)
            w2S = cp.tile([MLP_H, NF], FP16)
            zoS = cp.tile([1, MLP_H], FP16, name="zeros")
            nc.vector.memset(zoS[:], 0.0)
            zall = zp.tile([HID, (nstep + 1) * BL], F32)

            nc.sync.dma_start(out=b1S[:], in_=b1_in[:])
            nc.sync.dma_start(out=w1zS[:], in_=w1z_in[:])
            nc.sync.dma_start(out=w1aS[:], in_=w1a_in[:])
            nc.sync.dma_start(out=w1fS[:], in_=w1f_in[:])
            nc.sync.dma_start(out=s6S[:], in_=s6_in[:])
            nc.sync.dma_start(out=s3S[:], in_=s3_in[:])
            nc.sync.dma_start(out=w2S[:], in_=w2_in[:])
            if with_b2:
                b2cS = cp.tile([4, MLP_H], FP16)
                jselS = cp.tile([4, 4 * HB], FP16)
                nc.sync.dma_start(out=b2cS[:], in_=b2c_in[:])
                nc.sync.dma_start(out=jselS[:], in_=jsel_in[:])
            nc.sync.dma_start(out=zall[:, 0:BL], in_=z0t_in[:])
            nc.sync.dma_start(out=zs_out[:, 0:BL], in_=z0t_in[:])
            zh_prev = zhp.tile([HID, BL], FP16, name="zh0")
            nc.sync.dma_start(out=zh_prev[:], in_=z0h_in[:])
            zh_cur = zh_prev  # z_n fp16 for bases of stages 2..4

            kh_prev = None
            accP = None
            u_prev = [None] * NH
            sred_pending = [None] * NH
            for step in range(nstep):
                gslot = gp.tile([BL, 3 * BL], FP16, tag="g")
                nc.sync.dma_start(
                    out=gslot[:], in_=grep_in[:, step * 3 * BL : (step + 1) * 3 * BL]
                )
                accP = pa.tile([HID, BL], F32, tag="acc")
                # open the step's accumulation group over the full accP
                # region with one zeroing matmul; all S-red matmuls then
                # accumulate with start=False (two halves share the bank)
                nc.tensor.matmul(
                    accP[:],
                    lhsT=zoS[:, 0:HID],
                    rhs=zoS[:],
                    start=True,
                    stop=False,
                )
                for s in range(4):
                    col = step * 3 + CLS[s]
                    h_ps = []
                    hS = []
                    f_ps = []
                    fS = []
                    # PE: bases + chain mm1-accs, both halves
                    for h in range(NH):
                        bs = slice(h * HB, (h + 1) * HB)
                        hp_t = ph.tile([MLP_H, HB], F32, tag=f"hps{h}")
                        h_ps.append(hp_t)
                        if s == 0:
                            has_acc = kh_prev is not None
                            nc.tensor.matmul(
                                hp_t[:],
                                lhsT=w1zS[:],
                                rhs=zh_prev[:, bs],
                                start=True,
                                stop=not has_acc,
                            )
                            if has_acc:
                                nc.tensor.matmul(
                                    hp_t[:],
                                    lhsT=w1zS[:],
                                    rhs=kh_prev[:, bs],
                                    start=False,
                                    stop=True,
                                )
                        else:
                            wrep = w1fS if s == 3 else w1aS
                            nc.tensor.matmul(
                                hp_t[:],
                                lhsT=w1zS[:],
                                rhs=zh_cur[:, bs],
                                start=True,
                                stop=False,
                            )
                            for j in range(4):
                                nc.tensor.matmul(
                                    hp_t[:],
                                    lhsT=wrep[:, j * MLP_H : (j + 1) * MLP_H],
                                    rhs=u_prev[h][:, j * HB : (j + 1) * HB],
                                    start=False,
                                    stop=(j == 3),
                                )
                    # PE: side matmuls for the previous stage's z-update
                    for h in range(NH):
                        if sred_pending[h] is not None:
                            self_flush_sred(
                                nc, sred_pending[h], accP, h, HB, HID, False
                            )
                            sred_pending[h] = None
                    # DVE: relus
                    for h in range(NH):
                        hs_t = hp.tile([MLP_H, HB], FP16, tag=f"hs{h}")
                        hS.append(hs_t)
                        nc.vector.tensor_scalar(
                            hs_t[:],
                            h_ps[h][:],
                            b1S[:, col : col + 1],
                            0.0,
                            op0=mybir.AluOpType.add,
                            op1=mybir.AluOpType.max,
                        )
                    # PE: mm2 chunks
                    for h in range(NH):
                        fp_t = pf.tile([MLP_H, HNF], F32, tag=f"fps{h}")
                        f_ps.append(fp_t)
                        if with_b2:
                            nc.tensor.matmul(
                                fp_t[:],
                                lhsT=b2cS[:],
                                rhs=jselS[:],
                                start=True,
                                stop=False,
                            )
                        for j in range(4):
                            nc.tensor.matmul(
                                fp_t[:, j * HB : (j + 1) * HB],
                                lhsT=w2S[:, j * BL : (j + 1) * BL],
                                rhs=hS[h][:],
                                start=not with_b2,
                                stop=True,
                                skip_group_check=with_b2,
                            )
                    # ACT: tanh
                    for h in range(NH):
                        fs_t = fp.tile([MLP_H, HNF], FP16, tag=f"fs{h}")
                        fS.append(fs_t)
                        nc.scalar.activation(fs_t[:], f_ps[h][:], Tanh)
                    # DVE: mul by g
                    for h in range(NH):
                        u = up.tile([MLP_H, HNF], FP16, tag=f"u{h}")
                        u3 = u[:].rearrange("p (j b) -> p j b", j=4)
                        f3 = fS[h][:].rearrange("p (j b) -> p j b", j=4)
                        gv = (
                            gslot[:, CLS[s] * BL + h * HB : CLS[s] * BL + (h + 1) * HB]
                            .unsqueeze(1)
                            .broadcast_to((BL, 4, HB))
                        )
                        nc.vector.tensor_tensor(
                            out=u3, in0=f3, in1=gv, op=mybir.AluOpType.mult
                        )
                        sred_pending[h] = (
                            s6S if s in (0, 3) else s3S,
                            u,
                            s == 0,
                            s == 3,
                        )
                        u_prev[h] = u
                # flush stage-3 side matmuls now (kh depends on them)
                for h in range(NH):
                    self_flush_sred(
                        nc, sred_pending[h], accP, h, HB, HID, h == NH - 1
                    )
                    sred_pending[h] = None
                kh_prev = khp.tile([HID, BL], FP16, tag="kh")
                nc.vector.tensor_scalar_mul(kh_prev[:], accP[:], 1.0)
                zh_prev = zh_cur
                cur_sl = zall[:, step * BL : (step + 1) * BL]
                nxt_sl = zall[:, (step + 1) * BL : (step + 2) * BL]
                nc.vector.scalar_tensor_tensor(
                    out=nxt_sl,
                    in0=accP[:],
                    scalar=1.0,
                    in1=cur_sl,
                    op0=mybir.AluOpType.mult,
                    op1=mybir.AluOpType.add,
                )
                zh_cur = zhp.tile([HID, BL], FP16, tag="zh")
                nc.scalar.activation(zh_cur[:], nxt_sl, Copy)
                nc.gpsimd.dma_start(
                    out=zs_out[:, (step + 1) * BL : (step + 2) * BL], in_=nxt_sl
                )

    print(f"[kernel] tile trace+schedule: {_time.time()-t0:.1f}s", file=sys.stderr)
    t1 = _time.time()
    nc.finalize()
    print(f"[kernel] finalize: {_time.time()-t1:.1f}s", file=sys.stderr)
    return nc


def self_flush_sred(nc, pend, accP, h, HB, HID, last_half):
    sW, sU, s_start, s_stop = pend
    for j in range(4):
        nc.tensor.matmul(
            accP[:, h * HB : (h + 1) * HB],
            lhsT=sW[:, j * HID : (j + 1) * HID],
            rhs=sU[:, j * HB : (j + 1) * HB],
            start=False,
            stop=(s_stop and last_half and j == 3),
            skip_group_check=True,
        )


def _get_nc(nstep: int, with_b2: bool):
    key = (nstep, with_b2)
    if key not in _CACHE:
        _CACHE[key] = _build(nstep, with_b2)
    return _CACHE[key]


def _host_prep(coeffs, Wi1, bi1, Wi2, bi2, W1, b1, W2, b2, nstep: int):
    coeffs = np.asarray(coeffs, dtype=np.float32)
    a = coeffs[:, :, 0:8]
    b = coeffs[:, :, 8:16]
    c = coeffs[:, :, 16:24]
    d = coeffs[:, :, 24:32]

    X0 = a[:, 0]
    z0 = np.tanh(
        np.maximum(X0 @ Wi1 + bi1, 0.0).astype(np.float32) @ Wi2 + bi2
    ).astype(np.float32)

    # g[b, i, cls, c] = dX/dt at stage times (cls 0: t=i, 1: t=i+.5, 2: t=i+1)
    g = np.empty((B, nstep, 3, C_IN), dtype=np.float32)
    g[:, :, 0] = b[:, :nstep]
    g[:, :, 1] = b[:, :nstep] + c[:, :nstep] + 0.75 * d[:, :nstep]
    for i in range(nstep):
        if i + 1 < L - 1:
            g[:, i, 2] = b[:, i + 1]
        else:
            g[:, i, 2] = b[:, i] + 2.0 * c[:, i] + 3.0 * d[:, i]

    tcols = np.empty((nstep, 3), dtype=np.float32)
    tcols[:, 0] = np.arange(nstep, dtype=np.float32)
    tcols[:, 1] = tcols[:, 0] + 0.5
    tcols[:, 2] = tcols[:, 0] + 1.0
    bias1 = (
        b1[None, None, :] + tcols[:, :, None] * W1[0][None, None, :]
    ).astype(np.float32)
    bias1 = bias1.reshape(nstep * 3, MLP_H).T.copy()

    w1rep = np.repeat(np.asarray(W1[1:], np.float32), C_IN, axis=0)  # [512, 128]
    w1a = np.concatenate(
        [0.5 * w1rep[j * MLP_H : (j + 1) * MLP_H] for j in range(4)], axis=1
    )
    w1f = np.concatenate(
        [w1rep[j * MLP_H : (j + 1) * MLP_H] for j in range(4)], axis=1
    )
    sfull = np.repeat(np.eye(HID, dtype=np.float32), C_IN, axis=0)  # [512, 64]
    s6 = np.concatenate(
        [(1.0 / 6.0) * sfull[j * MLP_H : (j + 1) * MLP_H] for j in range(4)], axis=1
    )
    s3 = np.concatenate(
        [(1.0 / 3.0) * sfull[j * MLP_H : (j + 1) * MLP_H] for j in range(4)], axis=1
    )

    with_b2 = bool(np.any(np.asarray(b2)))
    shared = {
        "bias1": bias1,
        "w1z": np.ascontiguousarray(W1[1:], dtype=np.float16),
        "w1a": np.ascontiguousarray(w1a, dtype=np.float16),
        "w1f": np.ascontiguousarray(w1f, dtype=np.float16),
        "s6": np.ascontiguousarray(s6, dtype=np.float16),
        "s3": np.ascontiguousarray(s3, dtype=np.float16),
        "w2": np.ascontiguousarray(W2, dtype=np.float16),
    }
    if with_b2:
        shared["b2c"] = np.ascontiguousarray(
            np.asarray(b2, np.float32).reshape(4, MLP_H), dtype=np.float16
        )
        shared["jsel"] = np.ascontiguousarray(
            np.kron(np.eye(4, dtype=np.float32), np.ones((1, BL // NH), np.float32)),
            dtype=np.float16,
        )

    in_maps = []
    for core in range(NCORES):
        sl = slice(core * BL, (core + 1) * BL)
        m = dict(shared)
        gc = g[sl]  # [BL, nstep, 3, 8]
        arr = gc.transpose(3, 1, 2, 0)  # [8, nstep, 3, BL]
        rep = np.tile(arr, (MLP_H // C_IN, 1, 1, 1))  # [128, nstep, 3, BL]
        m["grep"] = np.ascontiguousarray(
            rep.reshape(MLP_H, nstep * 3 * BL), dtype=np.float16
        )
        z0t = np.ascontiguousarray(z0[sl].T)
        m["z0t"] = z0t
        m["z0h"] = np.ascontiguousarray(z0t, dtype=np.float16)
        in_maps.append(m)
    return in_maps, with_b2


def kernel(coeffs, Wi1, bi1, Wi2, bi2, W1, b1, W2, b2, _nstep: int = NSTEP,
           _trace: bool = False):
    import time as _time
    import sys

    nstep = _nstep
    in_maps, with_b2 = _host_prep(
        coeffs, Wi1, bi1, Wi2, bi2, W1, b1, W2, b2, nstep
    )
    nc = _get_nc(nstep, with_b2)
    t0 = _time.time()
    res = run_bass_kernel_spmd(nc, in_maps, list(range(NCORES)), trace=_trace)
    print(f"[kernel] spmd run (compile+exec): {_time.time()-t0:.1f}s", file=sys.stderr)
    out = np.empty((B, nstep + 1, HID), dtype=np.float32)
    for core in range(NCORES):
        zs = res.results[core]["zs"].reshape(HID, nstep + 1, BL)
        out[core * BL : (core + 1) * BL] = zs.transpose(2, 1, 0)
    if _trace:
        kernel.last_results = res
    return out
